# revision 2
# baseline (speedup 1.0000x reference)
"""AAEncoder (GNN message passing) on 8 NeuronCores.

Data-parallel over the hub-node axis N=1024: each of the 8 cores computes
128 rows of the N x N neighbor interaction + attention. The small parameter
set, the full neighbor position table (N*2 floats) and per-step velocities
are replicated on every device, so no cross-device communication is needed;
outputs are concatenated on the host.
"""
import numpy as np
import jax
import jax.numpy as jnp

N = 1024
D = 64
H = 8
DH = D // H
EPS = 1e-5
RADIUS = 50.0
M_DEV = 8
SHARD = N // M_DEV  # 128


def _ln(x, g, b):
    m = x.mean(-1, keepdims=True)
    v = ((x - m) ** 2).mean(-1, keepdims=True)
    return (x - m) / jnp.sqrt(v + EPS) * g + b


def _mm(x, W, b=None):
    """x @ W.T (+ b) with bf16 inputs, f32 accumulation — PE runs bf16 at
    4x the f32 column rate; LN right after each matmul renormalizes so the
    ~1e-3 bf16 rounding stays far below the 2e-2 gate."""
    y = jax.lax.dot_general(
        x.astype(jnp.bfloat16), W.astype(jnp.bfloat16).T,
        (((x.ndim - 1,), (0,)), ((), ())),
        preferred_element_type=jnp.float32)
    return y if b is None else y + b


def _shard_forward(i0, pos_t, dpos, pad_t, p):
    """Compute output rows [i0, i0+SHARD) of the encoder.

    pos_t, dpos: [N,2] replicated; pad_t: [N] bool replicated.
    """
    f32 = jnp.float32
    pos_i = jax.lax.dynamic_slice(pos_t, (i0, 0), (SHARD, 2))   # [S,2]
    dpos_i = jax.lax.dynamic_slice(dpos, (i0, 0), (SHARD, 2))   # [S,2]

    # cos/sin of atan2(dy,dx) without transcendentals: (dx,dy)/|dpos|
    rinv = jax.lax.rsqrt(dpos_i[:, 0] ** 2 + dpos_i[:, 1] ** 2)
    c, s = dpos_i[:, 0] * rinv, dpos_i[:, 1] * rinv             # [S]

    rel0 = pos_t[None, :, 0] - pos_i[:, 0:1]                    # [S,N]
    rel1 = pos_t[None, :, 1] - pos_i[:, 1:2]
    dist2 = rel0 * rel0 + rel1 * rel1
    col = jnp.arange(N)[None, :]
    row = i0 + jnp.arange(SHARD)[:, None]
    mask = (dist2 <= RADIUS * RADIUS) & (~pad_t)[None, :]
    mask = mask | (col == row)                                  # eye block
    # rel_rot[i,j,:] = R_i.T-free rotation: [rel.Rcol0, rel.Rcol1]
    rr0 = rel0 * c[:, None] + rel1 * s[:, None]                 # [S,N]
    rr1 = -rel0 * s[:, None] + rel1 * c[:, None]
    dpr0 = dpos_i[:, 0] * c + dpos_i[:, 1] * s                  # [S]
    dpr1 = -dpos_i[:, 0] * s + dpos_i[:, 1] * c
    dpos_rot = jnp.stack([dpr0, dpr1], -1)                      # [S,2]

    # center embedding + norm1 ([S,D] — tiny)
    h = dpos_rot[:, 0:1] * p['ce_W1'][:, 0] + dpos_rot[:, 1:2] * p['ce_W1'][:, 1] + p['ce_b1']
    h = jax.nn.relu(_ln(h, p['ce_g1'], p['ce_be1']))
    h = jax.nn.relu(_ln(_mm(h, p['ce_W2'], p['ce_b2']), p['ce_g2'], p['ce_be2']))
    center = _ln(_mm(h, p['ce_W3'], p['ce_b3']), p['ce_g3'], p['ce_be3'])
    center = _ln(center, p['n1_g'], p['n1_b'])                  # [S,D]

    # neighbor embedding, flat over pairs [S*N, D].
    # 2->D layer as broadcast mul-add (K=2 matmul is PE-hostile).
    e0p = (rr0.reshape(-1, 1) * p['ne0_W1'][:, 0] +
           rr1.reshape(-1, 1) * p['ne0_W1'][:, 1] + p['ne0_b1'])  # [S*N,D]
    e0 = _mm(jax.nn.relu(_ln(e0p, p['ne0_g1'], p['ne0_be1'])),
             p['ne0_W2'], p['ne0_b2'])                          # [S*N,D]
    # e1 depends only on the source node j -> [N,D], broadcast over hubs.
    e1p = dpos[:, 0:1] * p['ne1_W1'][:, 0] + dpos[:, 1:2] * p['ne1_W1'][:, 1] + p['ne1_b1']
    e1 = _mm(jax.nn.relu(_ln(e1p, p['ne1_g1'], p['ne1_be1'])),
             p['ne1_W2'], p['ne1_b2'])                          # [N,D]
    pre = e0.reshape(SHARD, N, D) + e1[None, :, :]
    nbr = _ln(_mm(jax.nn.relu(_ln(pre, p['na_g1'], p['na_be1'])).reshape(-1, D),
                  p['na_W'], p['na_b']),
              p['na_g2'], p['na_be2'])                          # [S*N,D]

    # single-query masked multihead attention per hub node.
    q = _mm(center, p['Wq']).reshape(SHARD, 1, H, DH)
    k = _mm(nbr, p['Wk']).reshape(SHARD, N, H, DH)
    v = _mm(nbr, p['Wv']).reshape(SHARD, N, H, DH)
    scale = 1.0 / np.sqrt(DH)
    logits = (q * k).sum(-1) * scale                            # [S,N,H]
    logits = jnp.where(mask[:, :, None], logits, -1e30)
    lmax = logits.max(axis=1, keepdims=True)                    # [S,1,H]
    pexp = jnp.exp(logits - lmax)                               # [S,N,H]
    den = pexp.sum(axis=1)                                      # [S,H]
    num = (pexp[..., None] * v).sum(axis=1)                     # [S,H,DH]
    mha = _mm((num / den[..., None]).reshape(SHARD, D), p['Wo'])

    gate = jax.nn.sigmoid(_mm(mha, p['ih_W'], p['ih_b']) +
                          _mm(center, p['hh_W'], p['hh_b']))
    out = mha + gate * (_mm(center, p['self_W'], p['self_b']) - mha)
    out = _ln(out, p['n2_g'], p['n2_b'])
    ff = _mm(jax.nn.relu(_mm(out, p['mlp_W1'], p['mlp_b1'])), p['mlp_W2'], p['mlp_b2'])
    return (out + ff).astype(f32)


_PMAPPED = None


def _get_pmapped():
    global _PMAPPED
    if _PMAPPED is None:
        _PMAPPED = jax.pmap(
            _shard_forward,
            in_axes=(0, None, None, None, None),
            static_broadcasted_argnums=(),
        )
    return _PMAPPED


def kernel(positions, bos_mask, padding_mask, t, params):
    del bos_mask  # unused by the math
    t = int(t)
    positions = np.asarray(positions, dtype=np.float32)
    pos_t = positions[:, t]                       # [N,2]
    dpos = positions[:, t] - positions[:, t - 1]  # [N,2]
    pad_t = np.asarray(padding_mask)[:, t]        # [N] bool
    p = {k: jnp.asarray(np.asarray(v)) for k, v in params.items()}

    i0s = jnp.arange(M_DEV, dtype=jnp.int32) * SHARD
    out = _get_pmapped()(i0s, jnp.asarray(pos_t), jnp.asarray(dpos),
                         jnp.asarray(pad_t), p)
    return np.asarray(out).reshape(N, D).astype(np.float32)


# revision 3
# speedup vs baseline: 1.2749x; 1.2749x over previous
"""AAEncoder (GNN message passing) on 8 NeuronCores.

Data-parallel over the hub-node axis N=1024: each of the 8 cores computes
128 rows of the N x N neighbor interaction + attention. The small parameter
set, the full neighbor position table (N*2 floats) and per-step velocities
are replicated on every device, so no cross-device communication is needed;
outputs are concatenated on the host.
"""
import numpy as np
import jax
import jax.numpy as jnp

N = 1024
D = 64
H = 8
DH = D // H
EPS = 1e-5
RADIUS = 50.0
M_DEV = 8
SHARD = N // M_DEV  # 128


def _ln(x, g, b):
    m = x.mean(-1, keepdims=True)
    v = ((x - m) ** 2).mean(-1, keepdims=True)
    return (x - m) / jnp.sqrt(v + EPS) * g + b


def _mm(x, W, b=None):
    """x @ W.T (+ b), f32."""
    y = jax.lax.dot_general(
        x, W.T, (((x.ndim - 1,), (0,)), ((), ())),
        preferred_element_type=jnp.float32)
    return y if b is None else y + b


def _shard_forward(i0, pos_t, dpos, pad_t, p):
    """Compute output rows [i0, i0+SHARD) of the encoder.

    pos_t, dpos: [N,2] replicated; pad_t: [N] bool replicated.
    """
    f32 = jnp.float32
    pos_i = jax.lax.dynamic_slice(pos_t, (i0, 0), (SHARD, 2))   # [S,2]
    dpos_i = jax.lax.dynamic_slice(dpos, (i0, 0), (SHARD, 2))   # [S,2]

    # cos/sin of atan2(dy,dx) without transcendentals: (dx,dy)/|dpos|
    rinv = jax.lax.rsqrt(dpos_i[:, 0] ** 2 + dpos_i[:, 1] ** 2)
    c, s = dpos_i[:, 0] * rinv, dpos_i[:, 1] * rinv             # [S]

    rel0 = pos_t[None, :, 0] - pos_i[:, 0:1]                    # [S,N]
    rel1 = pos_t[None, :, 1] - pos_i[:, 1:2]
    dist2 = rel0 * rel0 + rel1 * rel1
    col = jnp.arange(N)[None, :]
    row = i0 + jnp.arange(SHARD)[:, None]
    mask = (dist2 <= RADIUS * RADIUS) & (~pad_t)[None, :]
    mask = mask | (col == row)                                  # eye block
    # rel_rot[i,j,:] = R_i.T-free rotation: [rel.Rcol0, rel.Rcol1]
    rr0 = rel0 * c[:, None] + rel1 * s[:, None]                 # [S,N]
    rr1 = -rel0 * s[:, None] + rel1 * c[:, None]
    dpr0 = dpos_i[:, 0] * c + dpos_i[:, 1] * s                  # [S]
    dpr1 = -dpos_i[:, 0] * s + dpos_i[:, 1] * c
    dpos_rot = jnp.stack([dpr0, dpr1], -1)                      # [S,2]

    # center embedding + norm1 ([S,D] — tiny)
    h = dpos_rot[:, 0:1] * p['ce_W1'][:, 0] + dpos_rot[:, 1:2] * p['ce_W1'][:, 1] + p['ce_b1']
    h = jax.nn.relu(_ln(h, p['ce_g1'], p['ce_be1']))
    h = jax.nn.relu(_ln(_mm(h, p['ce_W2'], p['ce_b2']), p['ce_g2'], p['ce_be2']))
    center = _ln(_mm(h, p['ce_W3'], p['ce_b3']), p['ce_g3'], p['ce_be3'])
    center = _ln(center, p['n1_g'], p['n1_b'])                  # [S,D]

    # neighbor embedding, flat over pairs [S*N, D].
    # 2->D layer as broadcast mul-add (K=2 matmul is PE-hostile).
    e0p = (rr0.reshape(-1, 1) * p['ne0_W1'][:, 0] +
           rr1.reshape(-1, 1) * p['ne0_W1'][:, 1] + p['ne0_b1'])  # [S*N,D]
    e0 = _mm(jax.nn.relu(_ln(e0p, p['ne0_g1'], p['ne0_be1'])),
             p['ne0_W2'], p['ne0_b2'])                          # [S*N,D]
    # e1 depends only on the source node j -> [N,D], broadcast over hubs.
    e1p = dpos[:, 0:1] * p['ne1_W1'][:, 0] + dpos[:, 1:2] * p['ne1_W1'][:, 1] + p['ne1_b1']
    e1 = _mm(jax.nn.relu(_ln(e1p, p['ne1_g1'], p['ne1_be1'])),
             p['ne1_W2'], p['ne1_b2'])                          # [N,D]
    pre = e0.reshape(SHARD, N, D) + e1[None, :, :]
    nbr = _ln(_mm(jax.nn.relu(_ln(pre, p['na_g1'], p['na_be1'])).reshape(-1, D),
                  p['na_W'], p['na_b']),
              p['na_g2'], p['na_be2'])                          # [S*N,D]

    # single-query masked multihead attention per hub node.
    q = _mm(center, p['Wq']).reshape(SHARD, 1, H, DH)
    k = _mm(nbr, p['Wk']).reshape(SHARD, N, H, DH)
    v = _mm(nbr, p['Wv']).reshape(SHARD, N, H, DH)
    scale = 1.0 / np.sqrt(DH)
    logits = (q * k).sum(-1) * scale                            # [S,N,H]
    logits = jnp.where(mask[:, :, None], logits, -1e30)
    lmax = logits.max(axis=1, keepdims=True)                    # [S,1,H]
    pexp = jnp.exp(logits - lmax)                               # [S,N,H]
    den = pexp.sum(axis=1)                                      # [S,H]
    num = (pexp[..., None] * v).sum(axis=1)                     # [S,H,DH]
    mha = _mm((num / den[..., None]).reshape(SHARD, D), p['Wo'])

    gate = jax.nn.sigmoid(_mm(mha, p['ih_W'], p['ih_b']) +
                          _mm(center, p['hh_W'], p['hh_b']))
    out = mha + gate * (_mm(center, p['self_W'], p['self_b']) - mha)
    out = _ln(out, p['n2_g'], p['n2_b'])
    ff = _mm(jax.nn.relu(_mm(out, p['mlp_W1'], p['mlp_b1'])), p['mlp_W2'], p['mlp_b2'])
    return (out + ff).astype(f32)


_PMAPPED = None


def _get_pmapped():
    global _PMAPPED
    if _PMAPPED is None:
        _PMAPPED = jax.pmap(
            _shard_forward,
            in_axes=(0, None, None, None, None),
            static_broadcasted_argnums=(),
        )
    return _PMAPPED


def kernel(positions, bos_mask, padding_mask, t, params):
    del bos_mask  # unused by the math
    t = int(t)
    positions = np.asarray(positions, dtype=np.float32)
    pos_t = positions[:, t]                       # [N,2]
    dpos = positions[:, t] - positions[:, t - 1]  # [N,2]
    pad_t = np.asarray(padding_mask)[:, t]        # [N] bool
    p = {k: jnp.asarray(np.asarray(v)) for k, v in params.items()}

    i0s = jnp.arange(M_DEV, dtype=jnp.int32) * SHARD
    out = _get_pmapped()(i0s, jnp.asarray(pos_t), jnp.asarray(dpos),
                         jnp.asarray(pad_t), p)
    return np.asarray(out).reshape(N, D).astype(np.float32)


# revision 4
# speedup vs baseline: 1.5891x; 1.2464x over previous
"""AAEncoder (GNN message passing) on 8 NeuronCores.

Data-parallel over the hub-node axis N=1024: each of the 8 cores computes
128 rows of the N x N neighbor interaction + attention. The small parameter
set, the full neighbor position table (N*2 floats) and per-step velocities
are replicated on every device, so no cross-device communication is needed;
outputs are concatenated on the host.
"""
import numpy as np
import jax
import jax.numpy as jnp

N = 1024
D = 64
H = 8
DH = D // H
EPS = 1e-5
RADIUS = 50.0
M_DEV = 8
SHARD = N // M_DEV  # 128


def _ln(x, g, b):
    m = x.mean(-1, keepdims=True)
    v = ((x - m) ** 2).mean(-1, keepdims=True)
    return (x - m) / jnp.sqrt(v + EPS) * g + b


def _mm(x, W, b=None):
    """x @ W.T (+ b), f32."""
    y = jax.lax.dot_general(
        x, W.T, (((x.ndim - 1,), (0,)), ((), ())),
        preferred_element_type=jnp.float32)
    return y if b is None else y + b


def _shard_forward(i0, pos_t, dpos, pad_t, p):
    """Compute output rows [i0, i0+SHARD) of the encoder.

    pos_t, dpos: [N,2] replicated; pad_t: [N] bool replicated.
    """
    f32 = jnp.float32
    pos_i = jax.lax.dynamic_slice(pos_t, (i0, 0), (SHARD, 2))   # [S,2]
    dpos_i = jax.lax.dynamic_slice(dpos, (i0, 0), (SHARD, 2))   # [S,2]

    # cos/sin of atan2(dy,dx) without transcendentals: (dx,dy)/|dpos|
    rinv = jax.lax.rsqrt(dpos_i[:, 0] ** 2 + dpos_i[:, 1] ** 2)
    c, s = dpos_i[:, 0] * rinv, dpos_i[:, 1] * rinv             # [S]

    rel0 = pos_t[None, :, 0] - pos_i[:, 0:1]                    # [S,N]
    rel1 = pos_t[None, :, 1] - pos_i[:, 1:2]
    dist2 = rel0 * rel0 + rel1 * rel1
    col = jnp.arange(N)[None, :]
    row = i0 + jnp.arange(SHARD)[:, None]
    mask = (dist2 <= RADIUS * RADIUS) & (~pad_t)[None, :]
    mask = mask | (col == row)                                  # eye block
    # rel_rot[i,j,:] = R_i.T-free rotation: [rel.Rcol0, rel.Rcol1]
    rr0 = rel0 * c[:, None] + rel1 * s[:, None]                 # [S,N]
    rr1 = -rel0 * s[:, None] + rel1 * c[:, None]
    dpr0 = dpos_i[:, 0] * c + dpos_i[:, 1] * s                  # [S]
    dpr1 = -dpos_i[:, 0] * s + dpos_i[:, 1] * c
    dpos_rot = jnp.stack([dpr0, dpr1], -1)                      # [S,2]

    # center embedding + norm1 ([S,D] — tiny)
    h = dpos_rot[:, 0:1] * p['ce_W1'][:, 0] + dpos_rot[:, 1:2] * p['ce_W1'][:, 1] + p['ce_b1']
    h = jax.nn.relu(_ln(h, p['ce_g1'], p['ce_be1']))
    h = jax.nn.relu(_ln(_mm(h, p['ce_W2'], p['ce_b2']), p['ce_g2'], p['ce_be2']))
    center = _ln(_mm(h, p['ce_W3'], p['ce_b3']), p['ce_g3'], p['ce_be3'])
    center = _ln(center, p['n1_g'], p['n1_b'])                  # [S,D]

    # neighbor embedding, flat over pairs [S*N, D].
    # 2->D layer as broadcast mul-add (K=2 matmul is PE-hostile).
    e0p = (rr0.reshape(-1, 1) * p['ne0_W1'][:, 0] +
           rr1.reshape(-1, 1) * p['ne0_W1'][:, 1] + p['ne0_b1'])  # [S*N,D]
    e0 = _mm(jax.nn.relu(_ln(e0p, p['ne0_g1'], p['ne0_be1'])),
             p['ne0_W2'], p['ne0_b2'])                          # [S*N,D]
    # e1 depends only on the source node j -> [N,D], broadcast over hubs.
    e1p = dpos[:, 0:1] * p['ne1_W1'][:, 0] + dpos[:, 1:2] * p['ne1_W1'][:, 1] + p['ne1_b1']
    e1 = _mm(jax.nn.relu(_ln(e1p, p['ne1_g1'], p['ne1_be1'])),
             p['ne1_W2'], p['ne1_b2'])                          # [N,D]
    pre = e0.reshape(SHARD, N, D) + e1[None, :, :]
    nbr = _ln(_mm(jax.nn.relu(_ln(pre, p['na_g1'], p['na_be1'])).reshape(-1, D),
                  p['na_W'], p['na_b']),
              p['na_g2'], p['na_be2'])                          # [S*N,D]

    # single-query masked multihead attention per hub node.
    q = _mm(center, p['Wq']).reshape(SHARD, 1, H, DH)
    k = _mm(nbr, p['Wk']).reshape(SHARD, N, H, DH)
    v = _mm(nbr, p['Wv']).reshape(SHARD, N, H, DH)
    scale = 1.0 / np.sqrt(DH)
    logits = (q * k).sum(-1) * scale                            # [S,N,H]
    logits = jnp.where(mask[:, :, None], logits, -1e30)
    lmax = logits.max(axis=1, keepdims=True)                    # [S,1,H]
    pexp = jnp.exp(logits - lmax)                               # [S,N,H]
    den = pexp.sum(axis=1)                                      # [S,H]
    num = (pexp[..., None] * v).sum(axis=1)                     # [S,H,DH]
    mha = _mm((num / den[..., None]).reshape(SHARD, D), p['Wo'])

    gate = jax.nn.sigmoid(_mm(mha, p['ih_W'], p['ih_b']) +
                          _mm(center, p['hh_W'], p['hh_b']))
    out = mha + gate * (_mm(center, p['self_W'], p['self_b']) - mha)
    out = _ln(out, p['n2_g'], p['n2_b'])
    ff = _mm(jax.nn.relu(_mm(out, p['mlp_W1'], p['mlp_b1'])), p['mlp_W2'], p['mlp_b2'])
    return (out + ff).astype(f32)


_PMAPPED = None


def _get_pmapped():
    global _PMAPPED
    if _PMAPPED is None:
        _PMAPPED = jax.pmap(
            _shard_forward,
            in_axes=(0, None, None, None, None),
            static_broadcasted_argnums=(),
        )
    return _PMAPPED


def kernel(positions, bos_mask, padding_mask, t, params):
    del bos_mask  # unused by the math
    t = int(t)
    positions = np.asarray(positions, dtype=np.float32)
    pos_t = positions[:, t]                       # [N,2]
    dpos = positions[:, t] - positions[:, t - 1]  # [N,2]
    pad_t = np.asarray(padding_mask)[:, t]        # [N] bool
    p = {k: jnp.asarray(np.asarray(v), jnp.float32) for k, v in params.items()}

    if jax.device_count() >= M_DEV:
        i0s = jnp.arange(M_DEV, dtype=jnp.int32) * SHARD
        out = _get_pmapped()(i0s, jnp.asarray(pos_t), jnp.asarray(dpos),
                             jnp.asarray(pad_t), p)
    else:  # fallback: run the 8 shards sequentially on one device
        f = jax.jit(_shard_forward)
        out = np.stack([
            np.asarray(f(jnp.int32(c * SHARD), jnp.asarray(pos_t),
                         jnp.asarray(dpos), jnp.asarray(pad_t), p))
            for c in range(M_DEV)])
    return np.asarray(out).reshape(N, D).astype(np.float32)
